# revision 2
# baseline (speedup 1.0000x reference)
"""Trainium2 Bass kernel v2: batched RK4 of a tiny 2-4-1 LeakyReLU MLP ODE.

Algorithm (per element e, 99 RK4 steps, dt=1):
  dyn(s) = b2 + sum_j w_j * lrelu(a_j*s + c_je),  c_je = W1[1,j]*u_e + b1_j

V-space formulation: track V_j = a_j*(s + d_je) with d_je = c_je/a_j, so
lrelu-args are the V_j themselves.  Per RK4 stage i:
  r_i = lrelu(T_i)            (T_1 = V)
  T_{i+1} = V + q_i*(b2 + sum_j w_j r_ij)*a   (broadcast over j)
The j-sum+broadcast is one PE matmul with the constant 128x128 matrix
  B[ip,op] = w_{j(ip)} * a_{j(op)} * [ip%32 == op%32]
(partitions laid out as p = j*32 + ep; elements e = ep*512 + f).
The "+V + const*a" part is fused into a custom DVE op (two tensor streams),
so each stage is exactly:   1 matmul (PSUM)  +  1 fused DVE lrelu.
Final:  V' = BankF + V + b2*a  where BankF accumulates (1/6,2/6,2/6,1/6)-
weighted B@r_i matmuls.  s is recovered on host from the j=0 partitions:
s = V_0/a_0 - d_0.

Data-parallel over batch across 8 cores; 16384 elems/core as
[128 partitions = 4j x 32ep, 512 free].
"""

import sys
import numpy as np

sys.path.insert(0, "/opt/trn_rl_repo")

B = 131072
T = 100
NSTEP = 99
P = 128
NCORES = 8
PER = B // NCORES          # 16384 elements per core
EF = PER // P              # 512 free columns per core (4j x 32ep partitions)
EPB = 32                   # elements per partition-block
FREE = PER // EPB          # 512

CONFIG = {
    "G": 2,               # element groups (free-dim split); 512 % G == 0
    "stage_eng": ["dve", "dve", "dve"],   # engines for r2, r3, r4
    "r1_eng": "dve",      # 'dve' | 'act'
    "vp_eng": "act",      # V' materialize: 'dve' | 'act'
    "shared_bank": True,  # stages 2,3,4 share one PSUM bank per group
    "stagger": 0,         # dummy DVE ops delaying group 1's first step
    "interleave": False,  # emit stage-by-stage across groups
}


def _numpy_fallback(x, u, W1, b1, W2, b2):
    s = x[:, 0].astype(np.float32)
    uu = u[:, 0].astype(np.float32)
    traj = [s.copy()]
    for _ in range(NSTEP):
        def dyn(ss):
            z = np.stack([ss, uu], axis=-1)
            h = z @ W1 + b1
            h = np.where(h >= 0, h, np.float32(0.01) * h)
            return (h @ W2)[:, 0] + b2[0]
        k1 = dyn(s)
        k2 = dyn(s + np.float32(0.5) * k1)
        k3 = dyn(s + np.float32(0.5) * k2)
        k4 = dyn(s + k3)
        s = s + np.float32(1 / 6) * (k1 + 2 * k2 + 2 * k3 + k4)
        traj.append(s.copy())
    out = np.stack(traj, axis=1).astype(np.float32)
    return out[:, :, None]


_OPS_REGISTERED = {}


def _register_custom_ops():
    """Register the fused DVE ops (idempotent)."""
    if _OPS_REGISTERED:
        return _OPS_REGISTERED
    from concourse import dve_ops
    from concourse.dve_ops import DveOp, OPS, DveOpSpec, get_dve_sub_opcode, has_src1
    from concourse.dve_spec import Spec, Src0, Src1, C0, C1, maxx, lower

    def reg(name, spec, subdim=False):
        for op in OPS:
            if op.name == name:
                return op
        op = DveOp(name, spec, subdim=subdim, uops_sha={})
        OPS.append(op)
        dve_ops._SUB_OPCODE_FOR_NAME[name] = (
            dve_ops._CUSTOM_DVE_ROW_BASE + len(OPS) - 1)
        dve_ops.CUSTOM_DVE_SPECS[name] = spec
        for ver in ("v3", "v4"):
            compiled = DveOpSpec(
                name=name, opcode=get_dve_sub_opcode(name),
                uops=lower(spec, ver=ver), rd1_en=has_src1(spec))
            op.uops_sha[ver] = compiled.sha(ver)
        return op

    t_vs = Src0 + Src1 + C0
    lrelu_vs = reg("ANT_LRELU_VS", Spec(
        body=maxx(t_vs, t_vs * C1),
        reference=lambda in0, in1, s0, s1, imm2:
            np.maximum(in0 + in1 + s0, (in0 + in1 + s0) * s1)))
    t_s = Src0 + C0
    lrelu_s = reg("ANT_LRELU_S", Spec(
        body=maxx(t_s, t_s * C1),
        reference=lambda in0, in1, s0, s1, imm2:
            np.maximum(in0 + s0, (in0 + s0) * s1)))
    add_vs = reg("ANT_ADD_VS", Spec(
        body=Src0 + Src1 + C0,
        reference=lambda in0, in1, s0, s1, imm2: in0 + in1 + s0))
    _OPS_REGISTERED.update(
        lrelu_vs=lrelu_vs, lrelu_s=lrelu_s, add_vs=add_vs)
    return _OPS_REGISTERED


def _build_program(a4, w4, b2, cfg=None):
    """a4, w4: length-4 float lists; b2: float."""
    from concourse import bacc, tile, mybir
    from concourse.bass_types import AP

    ops = _register_custom_ops()
    cfg = dict(CONFIG, **(cfg or {}))
    G = cfg["G"]
    F = FREE // G
    AF = mybir.ActivationFunctionType
    f32 = mybir.dt.float32
    f32r = mybir.dt.float32r

    stage_eng = list(cfg["stage_eng"])

    nc = bacc.Bacc("TRN2", target_bir_lowering=False, debug=False)

    v0 = nc.dram_tensor("v0", [P, FREE], f32r, kind="ExternalInput")
    # stationary matrices, packed as [P, nW, 128]
    wnames = ["w05", "w1", "w16", "w26", "wid"]
    wmats = nc.dram_tensor("wmats", [P, len(wnames), P], f32r,
                           kind="ExternalInput")
    consts = nc.dram_tensor("consts", [P, 2], f32, kind="ExternalInput")
    out = nc.dram_tensor("out", [NSTEP, PER], f32, kind="ExternalOutput")

    import contextlib
    with tile.TileContext(nc) as tc, contextlib.ExitStack() as stk:
        pool = stk.enter_context(tc.tile_pool(name="main", bufs=1))
        ppool = stk.enter_context(tc.tile_pool(name="ps", bufs=1, space="PSUM"))

        Vb = [pool.tile([P, FREE], f32r, name=f"V{i}") for i in range(2)]
        W = {}
        wm = pool.tile([P, len(wnames), P], f32r, name="wm")
        cst = pool.tile([P, 2], f32, name="cst")
        nc.sync.dma_start(wm[:], wmats.ap())
        nc.sync.dma_start(cst[:], consts.ap())
        nc.sync.dma_start(Vb[0][:], v0.ap())
        for i, nm in enumerate(wnames):
            W[nm] = wm[:, i, :]
        c2ap = cst[:, 0:1]   # 0.5*b2*a_p
        c4ap = cst[:, 1:2]   # b2*a_p

        R = [[pool.tile([P, F], f32r, name=f"r{i}_{g}") for i in range(4)]
             for g in range(G)]
        if cfg["shared_bank"]:
            SB = [ppool.tile([P, 512], f32, name=f"sb_{g}") for g in range(G)]
            BF = [ppool.tile([P, 512], f32, name=f"bf_{g}") for g in range(G)]
            SBs = [SB[g][:, 0:F] for g in range(G)]
            BFs = [BF[g][:, 0:F] for g in range(G)]
        else:
            assert G == 2
            SB2 = [ppool.tile([P, 512], f32, name=f"sb2_{g}") for g in range(G)]
            SB3 = [ppool.tile([P, 512], f32, name=f"sb3_{g}") for g in range(G)]
            SB4 = [ppool.tile([P, 512], f32, name=f"sb4_{g}") for g in range(G)]
            BF = [ppool.tile([P, 512], f32, name=f"bf_{g}") for g in range(G)]
            BFs = [BF[g][:, 0:F] for g in range(G)]

        def fr(apv):
            return apv

        stage_w = ["w05", "w05", "w1"]       # weights for banks feeding r2,r3,r4
        bf_w = ["w16", "w26", "w26", "w16"]  # BankF accumulation weights

        def emit_nonlin(eng, dst, bank_ap, vcur, bias_ap):
            if eng == "dve":
                nc.vector._custom_dve(ops["lrelu_vs"], out=dst, in0=bank_ap,
                                      in1=vcur, s0=bias_ap, s1=0.01)
            else:
                nc.scalar.activation(dst, bank_ap, AF.Prelu,
                                     bias=bias_ap, scale=1.0, alpha=0.01)

        # r1 of step 0 from V0 directly; optionally stagger later groups by
        # a chain of dummy ops so group phases interleave on the engines
        for g in range(G):
            gs = slice(g * F, (g + 1) * F)
            delay = cfg.get("stagger", 0) * g
            if delay:
                dt_ = pool.tile([P, F], f32r, name=f"stg_{g}")
                nc.vector._custom_dve(ops["lrelu_s"], out=dt_[:],
                                      in0=Vb[0][:, gs], s0=0.0, s1=0.01)
                for _ in range(delay - 1):
                    nc.vector._custom_dve(ops["lrelu_s"], out=dt_[:],
                                          in0=dt_[:], s0=0.0, s1=0.01)
                nc.vector._custom_dve(ops["lrelu_s"], out=R[g][0][:],
                                      in0=dt_[:], s0=-1e30, s1=1.0)
            if not delay:
                nc.vector._custom_dve(ops["lrelu_s"], out=R[g][0][:],
                                      in0=Vb[0][:, gs], s0=0.0, s1=0.01)

        for step in range(NSTEP):
            cur_i, nxt_i = step % 2, (step + 1) % 2
            for g in range(G):
                gs = slice(g * F, (g + 1) * F)
                vcur = Vb[cur_i][:, gs]
                vnxt = Vb[nxt_i][:, gs]
                if not cfg["shared_bank"]:
                    sbanks = [SB2[g][:, 0:F], SB3[g][:, 0:F], SB4[g][:, 0:F]]
                else:
                    sbanks = [SBs[g]] * 3
                bfb = BFs[g]

                vp_act = cfg["vp_eng"] == "act"
                if vp_act:
                    # BF carries Id@V so ACT can materialize V' alone
                    nc.tensor.matmul(bfb, fr(W["wid"]), fr(vcur),
                                     start=True, stop=False)
                for i in range(3):       # stages producing r2, r3, r4
                    eng = stage_eng[i]
                    sb = sbanks[i]
                    # chain-critical stage mm first, BankF mm for the same r
                    # second (PE executes in order)
                    if eng == "act":
                        nc.tensor.matmul(sb, fr(W[stage_w[i]]), fr(R[g][i][:]),
                                         start=True, stop=False)
                        nc.tensor.matmul(sb, fr(W["wid"]), fr(vcur),
                                         start=False, stop=True)
                    else:
                        nc.tensor.matmul(sb, fr(W[stage_w[i]]), fr(R[g][i][:]),
                                         start=True, stop=True)
                    nc.tensor.matmul(bfb, fr(W[bf_w[i]]), fr(R[g][i][:]),
                                     start=(i == 0 and not vp_act), stop=False)
                    bias = c2ap if i < 2 else c4ap
                    emit_nonlin(eng, R[g][i + 1][:], sb,
                                None if eng == "act" else vcur, bias)
                # last BankF mm closes the accumulation group
                nc.tensor.matmul(bfb, fr(W[bf_w[3]]), fr(R[g][3][:]),
                                 start=False, stop=True)
                # next step's r1 = lrelu(V') straight from PSUM so it doesn't
                # wait for the V' materialization.
                if vp_act:
                    # BF already contains V
                    nc.vector._custom_dve(ops["lrelu_s"], out=R[g][0][:],
                                          in0=bfb, s0=c4ap, s1=0.01)
                    nc.scalar.activation(vnxt, bfb, AF.Identity,
                                         bias=c4ap, scale=1.0)
                else:
                    nc.vector._custom_dve(ops["lrelu_vs"], out=R[g][0][:],
                                          in0=bfb, in1=vcur, s0=c4ap, s1=0.01)
                    nc.vector._custom_dve(ops["add_vs"], out=vnxt, in0=bfb,
                                          in1=vcur, s0=c4ap)
            # DMA the j=0 slice of the new state: s-recoverable values
            src = Vb[nxt_i][0:EPB, :].bitcast(f32)
            out_ap = out.ap()
            dst = AP(out_ap.tensor, out_ap.offset + step * PER,
                     [[FREE, EPB], [1, FREE]])
            nc.sync.dma_start(dst, src)

    if not nc.is_finalized():
        nc.finalize()
    return nc


_PROGRAM_CACHE = {}


def _prep_core_inputs(x, u, W1, b1, W2, b2):
    a = W1[0, :].astype(np.float64)
    w = W2[:, 0].astype(np.float64)
    b2v = float(b2[0])
    cmat = (W1[1, :][None, :] * u[:, 0][:, None] + b1[None, :]).astype(np.float64)
    d = cmat / a[None, :]                       # [B, 4]
    y = x[:, 0].astype(np.float64)[:, None] + d  # [B, 4] y_j
    V0full = (a[None, :] * y)                    # [B, 4]

    Bmat = np.zeros((P, P), dtype=np.float64)
    for jin in range(4):
        for jout in range(4):
            blk = w[jin] * a[jout]
            for ep in range(EPB):
                Bmat[jin * EPB + ep, jout * EPB + ep] = blk

    avec = np.repeat(a, EPB)
    consts = np.stack([0.5 * b2v * avec, b2v * avec], axis=1).astype(np.float32)

    wd = {
        "w05": 0.5 * Bmat, "w1": Bmat,
        "w16": Bmat / 6.0, "w26": Bmat / 3.0,
        "wid": np.eye(P),
    }
    return a, d, V0full, wd, consts


def kernel(x, u, W1, b1, W2, b2, cfg=None):
    x = np.asarray(x, dtype=np.float32)
    u = np.asarray(u, dtype=np.float32)
    W1 = np.asarray(W1, dtype=np.float32)
    b1 = np.asarray(b1, dtype=np.float32)
    W2 = np.asarray(W2, dtype=np.float32)
    b2 = np.asarray(b2, dtype=np.float32)

    a_ = W1[0, :]
    if x.shape != (B, 1) or np.any(np.abs(a_) < 1e-6):
        return _numpy_fallback(x, u, W1, b1, W2, b2)

    from concourse import bass_utils

    cfg_t = tuple(sorted((dict(CONFIG, **(cfg or {}))).items(),
                         key=lambda kv: repr(kv[0])))
    key = repr(cfg_t)
    nc = _PROGRAM_CACHE.get(key)
    a, d, V0full, wd, consts = _prep_core_inputs(x, u, W1, b1, W2, b2)
    if nc is None:
        nc = _build_program([float(v) for v in a],
                            [float(v) for v in W2[:, 0]], float(b2[0]),
                            cfg=cfg)
        _PROGRAM_CACHE[key] = nc

    wnames = ["w05", "w1", "w16", "w26", "wid"]
    wpack = np.stack([wd[nm] for nm in wnames], axis=1).astype(np.float32)

    in_maps = []
    for cnum in range(NCORES):
        sl = slice(cnum * PER, (cnum + 1) * PER)
        # element e (local l) -> partition j*32 + (l // FREE), col l % FREE
        Vc = V0full[sl].reshape(EPB, FREE, 4)          # [ep, f, j]
        v0c = np.zeros((P, FREE), dtype=np.float32)
        for j in range(4):
            v0c[j * EPB:(j + 1) * EPB, :] = Vc[:, :, j]
        in_maps.append({"v0": np.ascontiguousarray(v0c),
                        "wmats": wpack, "consts": consts})

    res = bass_utils.run_bass_kernel_spmd(nc, in_maps, list(range(NCORES)))

    outf = np.empty((B, T), dtype=np.float32)
    outf[:, 0] = x[:, 0]
    inv_a0 = 1.0 / a[0]
    for cnum in range(NCORES):
        dev = np.asarray(res.results[cnum]["out"])     # [99, PER] V_0-values
        sl = slice(cnum * PER, (cnum + 1) * PER)
        s_traj = dev * np.float32(inv_a0) - d[sl, 0][None, :].astype(np.float32)
        outf[sl, 1:] = s_traj.T
    return outf[:, :, None].astype(np.float32)
